# revision 1
# baseline (speedup 1.0000x reference)
"""Trainium2 Bass kernel for nn_CompressiveEncoder (4-layer relative-position
transformer encoder, B=4 S=1024 D=1024 H=16 FF=4096).

Sharding: 8 cores = (batch b = c//2) x (query-half q0 = (c%2)*512).
Each layer starts with an 8-core AllGather of the bf16 transposed hidden
state; each core selects its batch pair from the gathered buffer via an
indirect DMA driven by host-supplied per-core indices, computes K/V for its
full batch, and attention + FFN for its own 512 query rows.  Matmuls run in
bf16 with fp32 PSUM accumulation; the residual stream stays fp32.  The
Music-Transformer shift() is a strided skewed DMA read from a DRAM scratch
buffer; the per-core query offset q0 is folded into a host-side roll of the
relative-position tables so the program is core-independent (SPMD).
"""

import sys

sys.path.insert(0, "/opt/trn_rl_repo")

import numpy as np
import ml_dtypes

import concourse.bass as bass
import concourse.mybir as mybir
import concourse.tile as tile
from concourse import bacc
from concourse.bass_utils import run_bass_kernel_spmd

BF16 = mybir.dt.bfloat16
F32 = mybir.dt.float32
AF = mybir.ActivationFunctionType
ALU = mybir.AluOpType

NL, D, H, DH, S, FF_DIM = 4, 1024, 16, 64, 1024, 4096
B = 4
NCORES = 8
NQ = 512           # query rows per core
P = 128
SCALE = float(H) ** -0.5   # reference scales by 1/sqrt(heads) = 0.25
EPS = 1e-5
NQC = NQ // P      # 4
NDC = D // P       # 8
NFC = FF_DIM // P  # 32
SKW = 2048         # skew buffer row width (elements)

_cache: dict = {}


def _ap(t, off, pattern):
    return bass.AP(tensor=t.tensor, offset=t.offset + off, ap=pattern)


def build():
    nc = bacc.Bacc("TRN2", target_bir_lowering=False, debug=False,
                   num_devices=NCORES)

    xrow0 = nc.dram_tensor("xrow0", [NQ, D], F32, kind="ExternalInput")
    xt0 = nc.dram_tensor("xt0", [D, NQ], BF16, kind="ExternalInput")
    wproj = nc.dram_tensor("wproj", [NL, 3, NDC, P, D], BF16,
                           kind="ExternalInput")
    wot = nc.dram_tensor("wot", [NL, D, D], BF16, kind="ExternalInput")
    w1r = nc.dram_tensor("w1r", [NL, NFC, P, D], BF16, kind="ExternalInput")
    w2t = nc.dram_tensor("w2t", [NL, FF_DIM, D], BF16, kind="ExternalInput")
    b1r = nc.dram_tensor("b1r", [NL, P, NFC], F32, kind="ExternalInput")
    b2r = nc.dram_tensor("b2r", [NL, D], F32, kind="ExternalInput")
    repd = nc.dram_tensor("repd", [NL, NDC, P, S], BF16,
                          kind="ExternalInput")
    rbd = nc.dram_tensor("rbd", [NL, H, S], BF16, kind="ExternalInput")
    rwbr = nc.dram_tensor("rwbr", [NL, P, NDC], F32, kind="ExternalInput")
    ident_d = nc.dram_tensor("ident", [P, P], BF16, kind="ExternalInput")
    agidx = nc.dram_tensor("agidx", [P, 2 * NDC], mybir.dt.int32,
                           kind="ExternalInput")
    yout = nc.dram_tensor("y", [NQ, D], F32, kind="ExternalOutput")

    # internal DRAM (raw tensors; indirect-DMA source needs offset 0)
    sk = nc.dram_tensor("sk_buf", [H * NQ * SKW], BF16)
    agin = nc.dram_tensor("agin_buf", [D * NQ], BF16)
    agout = nc.dram_tensor("agout_buf", [NCORES * D, NQ], BF16)

    with tile.TileContext(nc) as tc:
        with (
            tc.tile_pool(name="singles", bufs=1) as singles,
            tc.tile_pool(name="wT", bufs=2) as wTp,
            tc.tile_pool(name="wrhs", bufs=9) as wrhsp,
            tc.tile_pool(name="gt", bufs=17) as gtp,
            tc.tile_pool(name="attn", bufs=4) as attnp,
            tc.tile_pool(name="attnT", bufs=2) as attnTp,
            tc.tile_pool(name="brawsb", bufs=2) as brawp,
            tc.tile_pool(name="bd", bufs=2) as bdp,
            tc.tile_pool(name="rep", bufs=2) as repp,
            tc.tile_pool(name="small", bufs=16) as smallp,
            tc.tile_pool(name="vtt", bufs=2) as vttp,
            tc.tile_pool(name="rb", bufs=2) as rbp,
            tc.tile_pool(name="xw", bufs=2) as xwp,
            tc.tile_pool(name="psum", bufs=2, space="PSUM") as psp,
        ):
            # ------------- persistent SBUF state -------------
            x_row = [singles.tile([P, D], F32, tag=f"xrow{i}", name=f"xrow{i}")
                     for i in range(NQC)]
            xTown = [singles.tile([P, NQ], BF16, tag=f"xto{i}", name=f"xto{i}")
                     for i in range(NDC)]       # my own x^T (this layer's in)
            xT = [singles.tile([P, 2 * NQ], BF16, tag=f"xt{i}", name=f"xt{i}")
                  for i in range(NDC)]          # gathered x^T, my full batch
            xT1 = [singles.tile([P, NQ], BF16, tag=f"xt1_{i}", name=f"xt1_{i}")
                   for i in range(NDC)]         # post-LN1 x^T, my rows
            kT = [singles.tile([P, S], BF16, tag=f"kt{i}", name=f"kt{i}")
                  for i in range(NDC)]
            vrow = [singles.tile([P, D], BF16, tag=f"vr{i}", name=f"vr{i}")
                    for i in range(NDC)]
            rq = [singles.tile([P, NQ], BF16, tag=f"rq{i}", name=f"rq{i}")
                  for i in range(NDC)]
            aoT = [singles.tile([P, NQ], BF16, tag=f"aoT{i}", name=f"aoT{i}")
                   for i in range(NDC)]
            ff2acc = [singles.tile([P, D], F32, tag=f"ff2{i}", name=f"ff2{i}")
                      for i in range(NQC)]
            ident = singles.tile([P, P], BF16, tag="ident", name="ident")
            eps_t = singles.tile([P, 1], F32, tag="eps", name="eps")
            zb_t = singles.tile([P, 1], F32, tag="zbias", name="zbias")
            b2_t = singles.tile([P, D], F32, tag="b2rep", name="b2rep")
            b1_t = singles.tile([P, NFC], F32, tag="b1", name="b1")
            rwb_t = singles.tile([P, NDC], F32, tag="rwb", name="rwb")
            agix_t = singles.tile([P, 2 * NDC], mybir.dt.int32, tag="agix", name="agix")
            zeros_t = singles.tile([P, 1024], BF16, tag="zeros", name="zeros")

            nc.sync.dma_start(ident[:], ident_d.ap())
            nc.sync.dma_start(agix_t[:], agidx.ap())
            nc.vector.memset(eps_t[:], EPS)
            nc.vector.memset(zb_t[:], 0.0)
            nc.vector.memset(zeros_t[:], 0.0)
            skap = sk.ap()
            for blk in range(H * NQ // P):   # zero skew pad halves (once)
                dst = _ap(skap, blk * P * SKW + 1024, [[SKW, P], [1, 1024]])
                nc.sync.dma_start(dst, zeros_t[:, :])
            for qc in range(NQC):
                nc.sync.dma_start(x_row[qc][:],
                                  xrow0.ap()[qc * P:(qc + 1) * P, :])
            for dc in range(NDC):
                nc.sync.dma_start(xTown[dc][:],
                                  xt0.ap()[dc * P:(dc + 1) * P, :])
            nc.sync.dma_start(agin.ap(), xt0.ap())

            def mm(out, lhsT, rhs, first=True, last=True):
                nc.tensor.matmul(out, lhsT, rhs, start=first, stop=last)

            def layernorm(xr):
                st = smallp.tile([P, 2, 6], F32, tag="bnst", name="bnst")
                nc.vector.bn_stats(st[:, 0, :], xr[:, 0:512])
                nc.vector.bn_stats(st[:, 1, :], xr[:, 512:1024])
                mv = smallp.tile([P, 2], F32, tag="bnmv", name="bnmv")
                nc.vector.bn_aggr(mv[:], st[:])
                sd = smallp.tile([P, 1], F32, tag="sd", name="sd")
                nc.scalar.activation(sd[:], mv[:, 1:2], AF.Sqrt,
                                     bias=eps_t[:], scale=1.0)
                rs = smallp.tile([P, 1], F32, tag="rs", name="rs")
                nc.vector.reciprocal(rs[:], sd[:])
                nc.vector.tensor_scalar(
                    out=xr[:], in0=xr[:], scalar1=mv[:, 0:1],
                    scalar2=rs[:], op0=ALU.subtract, op1=ALU.mult)

            def transpose_to(dsts, src_bf16, qc):
                """src [128(q), 1024(d)] -> dsts[dc][:, qc*128:+128]."""
                for g in range(2):
                    pt = psp.tile([P, 512], BF16, tag="mm", name="mm")
                    for k4 in range(4):
                        dc = g * 4 + k4
                        nc.tensor.transpose(
                            pt[:, k4 * P:(k4 + 1) * P],
                            src_bf16[:, dc * P:(dc + 1) * P], ident[:])
                    for k4 in range(4):
                        dc = g * 4 + k4
                        nc.vector.tensor_copy(
                            dsts[dc][:, qc * P:(qc + 1) * P],
                            pt[:, k4 * P:(k4 + 1) * P])

            for li in range(NL):
                # ===== allgather x^T; pick my batch pair =====
                nc.gpsimd.collective_compute(
                    "AllGather", ALU.bypass,
                    replica_groups=[list(range(NCORES))],
                    ins=[agin.ap()], outs=[agout.ap()],
                )
                for dc in range(NDC):
                    for half in range(2):
                        nc.gpsimd.indirect_dma_start(
                            out=xT[dc][:, half * NQ:(half + 1) * NQ],
                            out_offset=None,
                            in_=agout.ap(),
                            in_offset=bass.IndirectOffsetOnAxis(
                                ap=agix_t[:, dc * 2 + half:
                                          dc * 2 + half + 1],
                                axis=0,
                            ),
                        )

                nc.sync.dma_start(rwb_t[:], rwbr.ap()[li])
                nc.sync.dma_start(b1_t[:], b1r.ap()[li])
                nc.sync.dma_start(
                    b2_t[:], _ap(b2r.ap(), li * D, [[0, P], [1, D]]))

                # ===== q/k/v projections =====
                for oc in range(NDC):
                    wq = wTp.tile([P, D], BF16, tag="wq", name="wq")
                    nc.sync.dma_start(wq[:], wproj.ap()[li, 0, oc])
                    ps = psp.tile([P, NQ], F32, tag="mm", name="mm")
                    for dc in range(NDC):
                        mm(ps[:], wq[:, dc * P:(dc + 1) * P], xTown[dc][:],
                           first=(dc == 0), last=(dc == NDC - 1))
                    nc.vector.tensor_scalar(
                        out=rq[oc][:], in0=ps[:],
                        scalar1=rwb_t[:, oc:oc + 1],
                        scalar2=None, op0=ALU.add)

                for oc in range(NDC):
                    wk = wTp.tile([P, D], BF16, tag="wq", name="wq")
                    nc.sync.dma_start(wk[:], wproj.ap()[li, 1, oc])
                    psk = psp.tile([P, S], F32, tag="score", name="score")
                    for jh in range(2):
                        for dc in range(NDC):
                            mm(psk[:, jh * 512:(jh + 1) * 512],
                               wk[:, dc * P:(dc + 1) * P],
                               xT[dc][:, jh * 512:(jh + 1) * 512],
                               first=(dc == 0), last=(dc == NDC - 1))
                    nc.vector.tensor_copy(kT[oc][:], psk[:])

                for oc in range(NDC):
                    wv = wTp.tile([P, D], BF16, tag="wq", name="wq")
                    nc.sync.dma_start(wv[:], wproj.ap()[li, 2, oc])
                    psv = psp.tile([P, S], F32, tag="score", name="score")
                    for jh in range(2):
                        for dc in range(NDC):
                            mm(psv[:, jh * 512:(jh + 1) * 512],
                               wv[:, dc * P:(dc + 1) * P],
                               xT[dc][:, jh * 512:(jh + 1) * 512],
                               first=(dc == 0), last=(dc == NDC - 1))
                    vt = vttp.tile([P, S], BF16, tag="vtt", name="vtt")
                    nc.vector.tensor_copy(vt[:], psv[:])
                    # transpose vT chunk into row-major v
                    for g in range(2):
                        pt = psp.tile([P, 512], BF16, tag="mm", name="mm")
                        for k4 in range(4):
                            sc = g * 4 + k4
                            nc.tensor.transpose(
                                pt[:, k4 * P:(k4 + 1) * P],
                                vt[:, sc * P:(sc + 1) * P], ident[:])
                        for k4 in range(4):
                            sc = g * 4 + k4
                            nc.vector.tensor_copy(
                                vrow[sc][:, oc * P:(oc + 1) * P],
                                pt[:, k4 * P:(k4 + 1) * P])

                # ===== attention =====
                for h in range(H):
                    ocn, rsub = h // 2, 64 * (h % 2)
                    if h % 2 == 0:
                        rept = repp.tile([P, S], BF16, tag="rep", name="rep")
                        nc.sync.dma_start(rept[:], repd.ap()[li, ocn])
                    rbt = rbp.tile([P, S], BF16, tag="rb", name="rb")
                    nc.sync.dma_start(
                        rbt[:], _ap(rbd.ap(), (li * H + h) * S,
                                    [[0, P], [1, S]]))
                    at_tiles = []
                    for qc in range(NQC):
                        psc = psp.tile([P, S], F32, tag="score", name="score")
                        psb = psp.tile([P, S], F32, tag="score", name="score")
                        for jh in range(2):
                            sl = slice(jh * 512, (jh + 1) * 512)
                            mm(psc[:, sl],
                               rq[ocn][rsub:rsub + 64, qc * P:(qc + 1) * P],
                               kT[ocn][rsub:rsub + 64, sl])
                            mm(psb[:, sl],
                               rq[ocn][rsub:rsub + 64, qc * P:(qc + 1) * P],
                               rept[rsub:rsub + 64, sl])
                        braw = brawp.tile([P, S], BF16, tag="braw", name="braw")
                        nc.vector.tensor_tensor(braw[:], psb[:], rbt[:],
                                                op=ALU.add)
                        base = h * NQ * SKW
                        nc.sync.dma_start(
                            _ap(skap, base + qc * P * SKW,
                                [[SKW, P], [1, 1024]]),
                            braw[:])
                        bdt = bdp.tile([P, S], BF16, tag="bd", name="bd")
                        nc.sync.dma_start(
                            bdt[:],
                            _ap(skap, base + qc * P * 2047 + 1023,
                                [[2047, P], [1, 1024]]))
                        nc.vector.tensor_tensor(psc[:], psc[:], bdt[:],
                                                op=ALU.add)
                        at = attnp.tile([P, S], BF16, tag="attn", name="attn")
                        zt = smallp.tile([P, 1], F32, tag="z", name="z")
                        nc.scalar.activation(at[:], psc[:], AF.Exp,
                                             bias=zb_t[:], scale=SCALE,
                                             accum_out=zt[:])
                        zr = smallp.tile([P, 1], F32, tag="zr", name="zr")
                        nc.vector.reciprocal(zr[:], zt[:])
                        nc.vector.tensor_scalar_mul(at[:], at[:], zr[:])
                        at_tiles.append(at)
                    pav = psp.tile([64, NQ], F32, tag="av", name="av")
                    for jc in range(NDC):
                        pt = psp.tile([P, 512], BF16, tag="mm", name="mm")
                        for qc in range(NQC):
                            nc.tensor.transpose(
                                pt[:, qc * P:(qc + 1) * P],
                                at_tiles[qc][:, jc * P:(jc + 1) * P],
                                ident[:])
                        atT = attnTp.tile([P, NQ], BF16, tag="atT", name="atT")
                        nc.vector.tensor_copy(atT[:], pt[:])
                        mm(pav[:], vrow[jc][:, h * DH:(h + 1) * DH], atT[:],
                           first=(jc == 0), last=(jc == NDC - 1))
                    tmpo = vttp.tile([64, NQ], BF16, tag="tmpo", name="tmpo")
                    nc.vector.tensor_copy(tmpo[:], pav[:])
                    nc.sync.dma_start(aoT[ocn][rsub:rsub + 64, :], tmpo[:])

                # ===== Wo + residual + LN1 + xT1 =====
                wo_t = []
                for dc in range(NDC):
                    w = wrhsp.tile([P, D], BF16, tag="wrhs", name="wrhs")
                    nc.sync.dma_start(
                        w[:], wot.ap()[li, dc * P:(dc + 1) * P, :])
                    wo_t.append(w)
                for qc in range(NQC):
                    for o2 in range(2):
                        sl = slice(o2 * 512, (o2 + 1) * 512)
                        pp = psp.tile([P, 512], F32, tag="mm", name="mm")
                        for dc in range(NDC):
                            mm(pp[:], aoT[dc][:, qc * P:(qc + 1) * P],
                               wo_t[dc][:, sl],
                               first=(dc == 0), last=(dc == NDC - 1))
                        nc.vector.tensor_add(x_row[qc][:, sl],
                                             x_row[qc][:, sl], pp[:])
                    layernorm(x_row[qc])
                    xb = xwp.tile([P, D], BF16, tag="xb", name="xb")
                    nc.vector.tensor_copy(xb[:], x_row[qc][:])
                    transpose_to(xT1, xb, qc)

                # ===== FFN =====
                for fh in range(2):
                    gts = []
                    for fc16 in range(16):
                        fc = fh * 16 + fc16
                        w1t_ = wTp.tile([P, D], BF16, tag="w1", name="w1")
                        nc.sync.dma_start(w1t_[:], w1r.ap()[li, fc])
                        ph = psp.tile([P, NQ], F32, tag="mm", name="mm")
                        for dc in range(NDC):
                            mm(ph[:], w1t_[:, dc * P:(dc + 1) * P],
                               xT1[dc][:],
                               first=(dc == 0), last=(dc == NDC - 1))
                        gt = gtp.tile([P, NQ], BF16, tag="gt", name="gt")
                        nc.scalar.activation(gt[:], ph[:], AF.Gelu,
                                             bias=b1_t[:, fc:fc + 1],
                                             scale=1.0)
                        gts.append(gt)
                    for fcg in range(2):
                        w2_t = []
                        for f8 in range(8):
                            w = wrhsp.tile([P, D], BF16, tag="wrhs",
                                           name="wrhs")
                            fc = fh * 16 + fcg * 8 + f8
                            nc.sync.dma_start(
                                w[:], w2t.ap()[li, fc * P:(fc + 1) * P, :])
                            w2_t.append(w)
                        for qc in range(NQC):
                            for o2 in range(2):
                                sl = slice(o2 * 512, (o2 + 1) * 512)
                                pf = psp.tile([P, 512], F32, tag="mm",
                                              name="mm")
                                for f8 in range(8):
                                    mm(pf[:],
                                       gts[fcg * 8 + f8][:,
                                                         qc * P:(qc + 1) * P],
                                       w2_t[f8][:, sl],
                                       first=(f8 == 0), last=(f8 == 7))
                                if fh == 0 and fcg == 0:
                                    nc.vector.tensor_add(ff2acc[qc][:, sl],
                                                         pf[:], b2_t[:, sl])
                                else:
                                    nc.vector.tensor_add(ff2acc[qc][:, sl],
                                                         ff2acc[qc][:, sl],
                                                         pf[:])

                # ===== residual + LN2; next-layer prep or output =====
                for qc in range(NQC):
                    nc.vector.tensor_add(x_row[qc][:], x_row[qc][:],
                                         ff2acc[qc][:])
                    layernorm(x_row[qc])
                    if li == NL - 1:
                        nc.sync.dma_start(
                            yout.ap()[qc * P:(qc + 1) * P, :], x_row[qc][:])
                    else:
                        xb = xwp.tile([P, D], BF16, tag="xb", name="xb")
                        nc.vector.tensor_copy(xb[:], x_row[qc][:])
                        transpose_to(xTown, xb, qc)
                if li < NL - 1:
                    for dc in range(NDC):
                        nc.sync.dma_start(
                            _ap(agin.ap(), dc * P * NQ, [[NQ, P], [1, NQ]]),
                            xTown[dc][:])

    nc.finalize()
    return nc


def _prep_host(inputs):
    bf = ml_dtypes.bfloat16
    embed = np.asarray(inputs["embed"], np.float32)
    seq = np.asarray(inputs["seq"]).astype(np.int64)
    x0 = embed[seq]                                   # [B, S, D] f32

    Wq = np.asarray(inputs["Wq"], np.float32)
    Wk = np.asarray(inputs["Wk"], np.float32)
    Wv = np.asarray(inputs["Wv"], np.float32)
    Wo = np.asarray(inputs["Wo"], np.float32)
    w1 = np.asarray(inputs["w1"], np.float32)
    w2 = np.asarray(inputs["w2"], np.float32)
    b1 = np.asarray(inputs["b1"], np.float32)
    b2 = np.asarray(inputs["b2"], np.float32)
    r_emb = np.asarray(inputs["r_emb"], np.float32)
    r_w_bias = np.asarray(inputs["r_w_bias"], np.float32)
    r_bias = np.asarray(inputs["r_bias"], np.float32)

    def packl(WT):   # [D, D] -> [NDC, P, D] lhsT pack
        return np.ascontiguousarray(
            WT.reshape(NDC, P, NDC, P).transpose(2, 1, 0, 3)
            .reshape(NDC, P, D))

    wproj = np.stack([
        np.stack([packl(Wq[l].T), packl(Wk[l].T), packl(Wv[l].T)])
        for l in range(NL)]).astype(bf)
    wot = np.stack([Wo[l].T for l in range(NL)]).astype(bf)
    w1r = np.stack([
        np.ascontiguousarray(
            w1[l].T.reshape(NDC, P, NFC, P).transpose(2, 1, 0, 3)
            .reshape(NFC, P, D))
        for l in range(NL)]).astype(bf)
    w2t = np.stack([w2[l].T for l in range(NL)]).astype(bf)
    b1r = np.stack([b1[l].reshape(NFC, P).T for l in range(NL)])
    b1r = np.ascontiguousarray(b1r).astype(np.float32)
    b2r = b2.astype(np.float32)
    rwbr = np.stack([r_w_bias[l].reshape(D).reshape(NDC, P).T
                     for l in range(NL)])
    rwbr = np.ascontiguousarray(rwbr).astype(np.float32)

    # rep: per head-pair stacked re^T; rb_adj = rb - rwb @ re^T separately
    off = r_emb.shape[2] - S     # MAX_KLEN - S
    rep = np.empty((NL, NDC, P, S), np.float32)
    rba = np.empty((NL, H, S), np.float32)
    for l in range(NL):
        for h in range(H):
            re = r_emb[l, h, off:, :]            # [S, DH]
            rep[l, h // 2, (h % 2) * 64:(h % 2) * 64 + 64] = re.T
            rba[l, h] = r_bias[l, h, off:] - r_w_bias[l, h] @ re.T

    ident = np.eye(P, dtype=bf)

    in_maps = []
    for c in range(NCORES):
        b, half = c // 2, c % 2
        q0 = half * NQ
        xr = np.ascontiguousarray(x0[b, q0:q0 + NQ]).astype(np.float32)
        xt = np.ascontiguousarray(x0[b, q0:q0 + NQ].T).astype(bf)
        repc = np.ascontiguousarray(np.roll(rep, q0, axis=-1)).astype(bf) \
            if q0 else rep.astype(bf)
        rbac = np.ascontiguousarray(np.roll(rba, q0, axis=-1)).astype(bf) \
            if q0 else rba.astype(bf)
        pvec = np.arange(P, dtype=np.int32)
        agix = np.empty((P, 2 * NDC), np.int32)
        for dc in range(NDC):
            for hh in range(2):
                agix[:, dc * 2 + hh] = (2 * b + hh) * D + dc * P + pvec
        in_maps.append({
            "xrow0": xr, "xt0": xt, "wproj": wproj, "wot": wot,
            "w1r": w1r, "w2t": w2t, "b1r": b1r, "b2r": b2r,
            "repd": repc, "rbd": rbac, "rwbr": rwbr, "ident": ident,
            "agidx": agix,
        })
    return in_maps


def run(inputs, trace=False):
    if "nc" not in _cache:
        _cache["nc"] = build()
    nc = _cache["nc"]
    in_maps = _prep_host(inputs)
    res = run_bass_kernel_spmd(nc, in_maps, list(range(NCORES)),
                               trace=trace)
    y = np.zeros((B, S, D), np.float32)
    for c in range(NCORES):
        b, half = c // 2, c % 2
        y[b, half * NQ:(half + 1) * NQ] = res.results[c]["y"]
    return y, res


def kernel(**inputs) -> np.ndarray:
    y, _ = run(inputs)
    return y


def timed_run(inputs, iters=3):
    """Correctness + device-exec timing: replicate run_bass_via_pjrt's
    multi-core path with inputs pre-staged on device."""
    import time
    import jax
    import jax.numpy as jnp
    from jax.sharding import Mesh, PartitionSpec
    from jax.experimental.shard_map import shard_map
    from concourse import bass2jax, mybir as _mb

    if "nc" not in _cache:
        _cache["nc"] = build()
    nc = _cache["nc"]
    in_maps = _prep_host(inputs)
    bass2jax.install_neuronx_cc_hook()

    partition_name = (nc.partition_id_tensor.name
                      if nc.partition_id_tensor else None)
    in_names, out_names, out_avals, zero_outs = [], [], [], []
    for alloc in nc.m.functions[0].allocations:
        if not isinstance(alloc, _mb.MemoryLocationSet):
            continue
        name = alloc.memorylocations[0].name
        if alloc.kind == "ExternalInput":
            if name != partition_name:
                in_names.append(name)
        elif alloc.kind == "ExternalOutput":
            out_names.append(name)
            shape = tuple(alloc.tensor_shape)
            dtype = _mb.dt.np(alloc.dtype)
            out_avals.append(jax.core.ShapedArray(shape, dtype))
            zero_outs.append(np.zeros(shape, dtype))
    n_params = len(in_names)
    n_outs = len(out_avals)
    all_in = list(in_names) + list(out_names)
    if partition_name is not None:
        all_in.append(partition_name)

    def _body(*args):
        operands = list(args)
        if partition_name is not None:
            operands.append(bass2jax.partition_id_tensor())
        outs = bass2jax._bass_exec_p.bind(
            *operands, out_avals=tuple(out_avals),
            in_names=tuple(all_in[:n_params] + out_names),
            out_names=tuple(out_names),
            lowering_input_output_aliases=(), sim_require_finite=True,
            sim_require_nnan=True, nc=nc)
        return tuple(outs)

    devices = jax.devices()[:NCORES]
    mesh = Mesh(np.asarray(devices), ("core",))
    in_specs = (PartitionSpec("core"),) * (n_params + n_outs)
    out_specs = (PartitionSpec("core"),) * n_outs
    fn = jax.jit(shard_map(_body, mesh=mesh, in_specs=in_specs,
                           out_specs=out_specs, check_rep=False),
                 keep_unused=True)
    concat_in = [np.concatenate([np.asarray(in_maps[c][nm])
                                 for c in range(NCORES)], axis=0)
                 for nm in in_names]
    concat_zeros = [np.zeros((NCORES * z.shape[0], *z.shape[1:]), z.dtype)
                    for z in zero_outs]
    staged = [jax.device_put(a) for a in concat_in + concat_zeros]
    out = fn(*staged)
    jax.block_until_ready(out)
    times = []
    for _ in range(iters):
        t0 = time.perf_counter()
        out = fn(*staged)
        jax.block_until_ready(out)
        times.append(time.perf_counter() - t0)
    y = np.zeros((B, S, D), np.float32)
    arr = np.asarray(out[out_names.index("y")]).reshape(NCORES, NQ, D)
    for c in range(NCORES):
        b_, half = c // 2, c % 2
        y[b_, half * NQ:(half + 1) * NQ] = arr[c]
    return y, min(times)

